# revision 1
# baseline (speedup 1.0000x reference)
"""Trainium2 Bass kernel for nn_MaxMarginLoss (segment_reduce).

Data-parallel over the batch: 32 samples -> 8 NeuronCores x 4 samples.

Per core, for each sample b:
  - segment sums over T=2048 timesteps into S=32 step buckets are computed
    on TensorE as mask[128t,32s].T @ |x|[128t,1024d], accumulated over 16
    K-chunks into PSUM (this is the memory-bound part: 32 MiB of `inputs`
    per core, streamed as 2 MiB contiguous DMAs).
  - the appearance-order logic avoids any sort: first-appearance positions
    come from a masked min-reduce; each step's rank is the count of
    strictly-smaller packed keys (pos*33 + id); the ordered-adjacency
    matrix A[i,j] = (rank_j == rank_i + 1 and j present) turns "gather by
    argsort and diff neighbours" into a tiny 32x32 matmul H_next = A @ H.
  - pair energies E_i = mean_d relu(H_i - H_next_i)^2 via Relu + Square
    with fused free-dim accumulation.
Each core returns [4,5] per-sample sums (npairs, n, ninv, sum E*valid,
sum relu(1-E)*inv); the host applies the binary labels and the final
scalar division (a few hundred flops).
"""

import numpy as np

import concourse.bass as bass
from concourse import mybir
from concourse.bass_utils import run_bass_kernel_spmd
from concourse.tile import TileContext
from concourse.vector_clock import ScopedClock

F32 = mybir.dt.float32
BF16 = mybir.dt.bfloat16
U32 = mybir.dt.uint32
U16 = mybir.dt.uint16
I8 = mybir.dt.int8
I16 = mybir.dt.int16
OP = mybir.AluOpType
AF = mybir.ActivationFunctionType

B, T, D = 32, 2048, 1024
S = 32          # step ids 1..32; id 0 is padding
ALPHA = 1.0
N_CORES = 8
BL = B // N_CORES           # samples per core
K = 128                     # matmul contraction tile (partitions)
NCHUNK = T // K             # 16 K-chunks per sample
XT = 2                      # K-chunks per x DMA ([128, XT, 1024] = 1 MiB)

# The public neuronxcc walrus (setupSyncWait in CoreV2/V3GenImpl) only
# supports a small number of embedded semaphore waits per instruction,
# while Tile's scheduler attaches one wait per required logical proc.
# After scheduling, hoist overflow waits onto same-engine no-ops placed
# immediately before the owning instruction: engine program order makes
# that semantically identical.
_MAX_WAITS_DEFAULT = 1
_MAX_WAITS_BY_OPCODE = {}


class _LeanTailTileContext(TileContext):
    """Tile's default kernel tail is drain -> barrier -> sem-clear ->
    barrier.  After the first all-engine barrier no engine can still be
    waiting on a kernel semaphore, so the clears need no cross-engine
    ordering and the second (~3-4 us) barrier can be dropped; each
    engine's stream still ends after its own clears, so re-execution
    sees zeroed semaphores."""

    def _drain_and_barrier(self, tick_clock, wait_clock):
        drain_inst = self.nc.sync.drain()
        wait_clock.add_sem_waits(
            drain_inst.ins, ScopedClock({None: tick_clock.global_clock})
        )
        self.nc.all_engine_barrier()
        assert self.sems is not None
        popped = self.nc._tile_sem_poison_stack.pop()
        assert popped is self._sem_poison
        self.nc.clear_and_free_semaphores(list(self.sems.allocated().values()))


def _split_sync_waits(nc: bass.Bass):
    for f in nc.m.functions:
        for bb in f.blocks:
            insts = list(bb.instructions)
            need = []  # (ins, overflow_waits)
            for ins in insts:
                si = getattr(ins, "sync_info", None)
                if si is None or not si.on_wait:
                    continue
                cap = _MAX_WAITS_BY_OPCODE.get(ins.opcode, _MAX_WAITS_DEFAULT)
                waits = list(si.on_wait)
                if len(waits) <= cap:
                    continue
                ins.sync_info = mybir.SyncInfo(
                    on_wait=waits[:cap], on_update=list(si.on_update)
                )
                need.append((ins, waits[cap:], cap))
            if not need:
                continue
            nop_for: dict[str, list] = {}
            for ins, overflow, cap in need:
                eng = nc.engines[ins.engine]
                nops = []
                for i in range(0, len(overflow), cap):
                    nop = eng.nop(hint="waitsplit", nofuse=True)
                    nop.ins.sync_info = mybir.SyncInfo(
                        on_wait=overflow[i:i + cap], on_update=[]
                    )
                    nops.append(nop.ins)
                nop_for[ins.name] = nops
            created = {n.name for nops in nop_for.values() for n in nops}
            # nop() appended the new instructions to the current bb; pull
            # them out of every block and splice before their owners.
            for bb2 in f.blocks:
                cur = [i for i in bb2.instructions if i.name not in created]
                out = []
                for ins in cur:
                    out.extend(nop_for.get(ins.name, ()))
                    out.append(ins)
                bb2.instructions = out


# column offsets inside the packed int8 index buffer "cst8"
C8_IDSBC = 0                  # [K, T]   step ids, row 32b+s = ids of sample b
C8_IDSREP = C8_IDSBC + T      # [K, BL*NCHUNK*S] mask-layout ids, 32x repeated
C8_IOTAT = C8_IDSREP + BL * NCHUNK * S   # [K, NCHUNK*S] tiled 1..32
CW8 = C8_IOTAT + NCHUNK * S
# column offsets inside the packed float32 constant buffer "cst32"
C_STEPS = 0                   # [K, 1]
C_LOWER = C_STEPS + 1         # [K, S] block [i > j]
C_ONES = C_LOWER + S          # [K, S] ones
C_BONES = C_ONES + S          # [K, BL] block-diagonal ones
CW32 = C_BONES + BL


def build_program() -> bass.Bass:
    nc = bass.Bass()

    x = nc.declare_dram_parameter("x", [BL, T, D], F32, isOutput=False)
    cst8 = nc.declare_dram_parameter("cst8", [K, CW8], I8, isOutput=False)
    tmt16 = nc.declare_dram_parameter("tmt16", [K, T], I16, isOutput=False)
    cst32 = nc.declare_dram_parameter("cst32", [K, CW32], F32, isOutput=False)
    out5 = nc.declare_dram_parameter("out5", [BL, 5], F32, isOutput=True)

    with _LeanTailTileContext(nc) as tc:
        with (
            tc.tile_pool(name="const", bufs=1) as cpool,
            tc.tile_pool(name="persist", bufs=1) as pp,
            tc.tile_pool(name="xin", bufs=12) as xin,
            tc.tile_pool(name="xabs", bufs=12) as xabs,
            tc.tile_pool(name="mk", bufs=2) as mkp,
            tc.tile_pool(name="ps_sums", bufs=2, space="PSUM") as ps_sums,
            tc.tile_pool(name="ps_misc", bufs=1, space="PSUM") as ps_misc,
        ):
            # ---- constants / index data, narrow dtypes, DMA'd via the
            #      (otherwise idle) SWDGE queue so the HWDGE rings start
            #      on x immediately --------------------------------------
            sb_cst8 = cpool.tile([K, CW8], I8)
            nc.gpsimd.dma_start(out=sb_cst8[:], in_=cst8[:])
            sb_tmt = cpool.tile([K, T], I16)
            nc.gpsimd.dma_start(out=sb_tmt[:], in_=tmt16[:])
            sb_cst32 = cpool.tile([K, CW32], F32)
            nc.gpsimd.dma_start(out=sb_cst32[:], in_=cst32[:])
            sb_idsbc = sb_cst8[:, C8_IDSBC:C8_IDSBC + T]
            sb_steps = sb_cst32[:, C_STEPS:C_STEPS + 1]
            sb_lower = sb_cst32[:, C_LOWER:C_LOWER + S]
            sb_ones = sb_cst32[:, C_ONES:C_ONES + S]
            sb_bones = sb_cst32[:, C_BONES:C_BONES + BL]

            # ---- phase A: masks / positions / ranks (all 4 samples
            #      stacked on partitions: row 32*b + s) ------------------
            maskf = pp.tile([K, T], F32)        # [s-stacked, t] 0/1 mask
            counts = pp.tile([K, 1], F32)
            nc.vector.tensor_scalar(
                maskf[:], sb_idsbc[:], sb_steps[:], None, OP.is_equal, OP.add,
                accum_out=counts[:],
            )
            tm = pp.tile([K, T], F32)           # mask * (t - T)
            nc.vector.tensor_tensor(tm[:], maskf[:], sb_tmt[:], OP.mult)
            posm = pp.tile([K, 1], F32)         # pos - T (present) else 0
            nc.vector.tensor_reduce(posm[:], tm[:], mybir.AxisListType.X, OP.min)

            cnt1 = pp.tile([K, 1], F32)
            nc.vector.tensor_scalar(cnt1[:], counts[:], 1.0, None, OP.max)
            recip = pp.tile([K, 1], F32)        # 1 / max(counts, 1)
            nc.vector.reciprocal(recip[:], cnt1[:])

            # distinct sort keys: (pos-T)*33 + (s+1); order == stable
            # argsort of pos with id tiebreak (present strictly first)
            key = pp.tile([K, 1], F32)
            nc.vector.tensor_scalar(
                key[:], posm[:], 33.0, sb_steps[:], OP.mult, OP.add
            )
            key_sq = pp.tile([K, S], F32)
            nc.vector.tensor_scalar(key_sq[:], sb_ones[:], key[:], None, OP.mult)
            key_t = pp.tile([K, S], F32)        # row i holds key_l along l
            nc.vector.transpose(key_t[:], key_sq[:])
            cmp = pp.tile([K, S], F32)
            rank = pp.tile([K, 1], F32)
            nc.vector.tensor_scalar(
                cmp[:], key_t[:], key[:], None, OP.is_lt, OP.add,
                accum_out=rank[:],
            )
            rankp1 = pp.tile([K, 1], F32)
            nc.vector.tensor_scalar(rankp1[:], rank[:], 1.0, None, OP.add)
            t999 = pp.tile([K, 1], F32)         # 999 for absent steps
            nc.vector.tensor_scalar(
                t999[:], posm[:], 0.0, 999.0, OP.is_ge, OP.mult
            )
            rankp = pp.tile([K, 1], F32)        # rank, pushed out if absent
            nc.vector.tensor_tensor(rankp[:], rank[:], t999[:], OP.add)

            v_t = pp.tile([K, 8], F32)          # per-step stats columns
            nc.vector.tensor_scalar(v_t[:, 1:2], posm[:], 0.0, None, OP.is_lt)

            rankp_sq = pp.tile([K, S], F32)
            nc.vector.tensor_scalar(rankp_sq[:], sb_ones[:], rankp[:], None, OP.mult)
            rankp_t = pp.tile([K, S], F32)
            nc.vector.transpose(rankp_t[:], rankp_sq[:])
            rankp1_sq = pp.tile([K, S], F32)
            nc.vector.tensor_scalar(rankp1_sq[:], sb_ones[:], rankp1[:], None, OP.mult)
            rankp1_t = pp.tile([K, S], F32)
            nc.vector.transpose(rankp1_t[:], rankp1_sq[:])

            # A[i,j] = (rankp_j == rank_i + 1); succ_i = sum_j A[i,j]
            a_m = pp.tile([K, S], F32)
            nc.vector.tensor_scalar(
                a_m[:], rankp_t[:], rankp1[:], None, OP.is_equal, OP.add,
                accum_out=v_t[:, 0:1],
            )
            # A^T (lhsT for the H_next matmul; 0/1 so bf16 is exact)
            a_t = pp.tile([K, S], BF16)
            nc.vector.tensor_scalar(
                a_t[:], rankp1_t[:], rankp[:], None, OP.is_equal
            )
            # inv_i = sum_j A[i,j] * [i > j]
            a_inv = pp.tile([K, S], F32)
            nc.vector.scalar_tensor_tensor(
                a_inv[:], rankp_t[:], rankp1[:], sb_lower[:],
                op0=OP.is_equal, op1=OP.mult, accum_out=v_t[:, 2:3],
            )

            # ---- phase B: segment sums via TensorE --------------------
            h_all = pp.tile([K, D], BF16)
            hn = ps_misc.tile([K, D], F32)      # 2 PSUM banks
            diff = pp.tile([K, D], F32)
            sq = pp.tile([K, D], F32)
            e_raw = pp.tile([K, 1], F32)
            ps_of = {}

            # Per-sample tail, emitted one sample late (during sample b+1's
            # stream) so the ops land in each engine's in-order queue at a
            # point where their dependencies are already met — emitted
            # eagerly they head-of-line-block the abs stream and stall the
            # DMAs.  The H-scale half runs as early as possible because it
            # releases sample b's PSUM banks for sample b+2.
            def sample_scale(b):
                # on ACT (activation Copy with per-partition scale) so the
                # tail's critical path doesn't serialize behind the DVE abs
                ps_all = ps_of[b]
                bs = slice(b * S, (b + 1) * S)
                for h in range(2):
                    nc.scalar.activation(
                        h_all[bs, h * 512:(h + 1) * 512],
                        ps_all[bs, h * 512:(h + 1) * 512],
                        AF.Copy, scale=recip[bs],
                    )

            def sample_tail(b):
                bs = slice(b * S, (b + 1) * S)
                for h in range(2):
                    nc.tensor.matmul(
                        hn[bs, h * 512:(h + 1) * 512],
                        lhsT=a_t[bs, :],
                        rhs=h_all[bs, h * 512:(h + 1) * 512],
                        start=True, stop=True,
                        tile_position=(b * S, b * S),
                    )
                nc.vector.tensor_tensor(
                    diff[bs, :], h_all[bs, :], hn[bs, :], OP.subtract
                )
                # relu(d)^2 == max(d,0)*d, with the free-dim sum fused in
                nc.vector.scalar_tensor_tensor(
                    sq[bs, :], diff[bs, :], 0.0, diff[bs, :],
                    op0=OP.max, op1=OP.mult, accum_out=e_raw[bs, :],
                )

            for b in range(BL):
                # all 16 mask chunks of the sample in one compare against
                # host-replicated ids (layout matches the x DMA below)
                mk_all = mkp.tile([K, NCHUNK * S], BF16)
                nc.vector.tensor_tensor(
                    mk_all[:],
                    sb_cst8[:, C8_IDSREP + b * NCHUNK * S:
                            C8_IDSREP + (b + 1) * NCHUNK * S],
                    sb_cst8[:, C8_IOTAT:C8_IOTAT + NCHUNK * S],
                    OP.is_equal,
                )
                # fresh PSUM banks per sample: sample b+1 accumulates while
                # sample b's H-scale still reads its own banks (no WAR)
                ps_all = ps_sums.tile([K, D], F32, tag="ps")
                ps_of[b] = ps_all
                for tq in range(NCHUNK // XT):
                    if tq == 1 and b > 0:
                        sample_scale(b - 1)
                    if tq == (NCHUNK // XT) // 2 and b > 0:
                        sample_tail(b - 1)
                    ti = b * (NCHUNK // XT) + tq
                    xt = xin.tile([K, XT, D], F32)
                    # All x DMAs go through the sync ring: the scalar ring's
                    # issue ops share the ACT sequencer with the abs
                    # ACTIVATEs, so a data-starved abs head-of-line-blocks
                    # later DMA issues and stalls the stream.  With 16 KiB
                    # of contiguous DRAM per partition (XT consecutive
                    # T-rows per partition; any (partition, sub) <-> t
                    # bijection works as long as the host ids layout
                    # matches), one ring's descriptor feed saturates all 16
                    # SDMA engines.
                    dma_eng = nc.sync
                    dma_eng.dma_start(
                        out=xt[:],
                        in_=x[b, tq * XT * K:(tq + 1) * XT * K, :].rearrange(
                            "(p s) d -> p s d", p=K
                        ),
                    )
                    # |x| rounded to bf16: the PE runs bf16 at 1 cycle/row
                    # vs fp32's 4; the 2^-9 relative rounding on |x| washes
                    # out to ~1e-4 in the final loss (mask stays exact 0/1).
                    # Alternate engines: ACT computes Abs->bf16 directly; DVE
                    # rounds to bf16 (RNE, so |bf16(x)| == bf16(|x|)) then
                    # clears the sign bit in place in the 16-bit 4x mode.
                    xa = xabs.tile([K, XT, D], BF16)
                    if ti % 2 == 0:
                        nc.scalar.activation(xa[:], xt[:], AF.Abs)
                    else:
                        nc.vector.tensor_copy(xa[:], xt[:])
                        nc.vector.tensor_scalar(
                            xa[:].bitcast(U16), xa[:].bitcast(U16),
                            0x7FFF, None, OP.bitwise_and,
                        )
                    for sub in range(XT):
                        c = tq * XT + sub
                        for h in range(2):
                            nc.tensor.matmul(
                                ps_all[b * S:(b + 1) * S, h * 512:(h + 1) * 512],
                                lhsT=mk_all[:, c * S:(c + 1) * S],
                                rhs=xa[:, sub, h * 512:(h + 1) * 512],
                                start=(c == 0), stop=(c == NCHUNK - 1),
                                tile_position=(0, b * S),
                            )
            sample_scale(BL - 1)
            sample_tail(BL - 1)

            # ---- phase C: combine per-step stats ----------------------
            e_col = pp.tile([K, 1], F32)
            nc.vector.tensor_scalar(e_col[:], e_raw[:], 1.0 / D, None, OP.mult)
            nc.vector.tensor_tensor(v_t[:, 3:4], e_col[:], v_t[:, 0:1], OP.mult)
            ae1 = pp.tile([K, 1], F32)          # relu(ALPHA - E)
            nc.vector.tensor_scalar(
                ae1[:], e_col[:], -1.0, ALPHA, OP.mult, OP.add
            )
            ae = pp.tile([K, 1], F32)
            nc.vector.tensor_scalar(ae[:], ae1[:], 0.0, None, OP.max)
            nc.vector.tensor_tensor(v_t[:, 4:5], ae[:], v_t[:, 2:3], OP.mult)

            # per-sample column sums: blockones[128,4].T @ V[128,5] -> [4,5]
            vp = ps_misc.tile([BL, 8], F32)
            nc.tensor.matmul(
                vp[:, 0:5], lhsT=sb_bones[:], rhs=v_t[:, 0:5],
                start=True, stop=True,
            )
            out_sb = pp.tile([BL, 5], F32)
            nc.vector.tensor_copy(out_sb[:], vp[:, 0:5])
            nc.sync.dma_start(out=out5[:], in_=out_sb[:])

    _split_sync_waits(nc)
    return nc


_PROGRAM: bass.Bass | None = None


def get_program() -> bass.Bass:
    global _PROGRAM
    if _PROGRAM is None:
        _PROGRAM = build_program()
    return _PROGRAM


def make_in_maps(inputs: np.ndarray, step_ids: np.ndarray) -> list[dict]:
    """Shard + pre-layout the (tiny) index tensors per core."""
    inputs = np.ascontiguousarray(np.asarray(inputs, dtype=np.float32))
    step_ids = np.asarray(step_ids)

    tmt16 = np.tile(
        (np.arange(T) - T).astype(np.int16)[None, :], (K, 1)
    )
    iota_t = np.tile(
        np.tile(np.arange(1, S + 1, dtype=np.int8), NCHUNK)[None, :], (K, 1)
    )
    cst32 = np.empty((K, CW32), dtype=np.float32)
    cst32[:, C_STEPS:C_STEPS + 1] = np.tile(
        np.arange(1, S + 1, dtype=np.float32), BL
    )[:, None]
    cst32[:, C_LOWER:C_LOWER + S] = np.tile(
        (np.arange(S)[:, None] > np.arange(S)[None, :]).astype(np.float32),
        (BL, 1),
    )
    cst32[:, C_ONES:C_ONES + S] = 1.0
    cst32[:, C_BONES:C_BONES + BL] = (
        (np.arange(K)[:, None] // S) == np.arange(BL)[None, :]
    ).astype(np.float32)

    in_maps = []
    for core in range(N_CORES):
        b0 = core * BL
        ids = step_ids[b0:b0 + BL].astype(np.int8)              # [4, 2048]
        # matmul chunk (b, tq, sub) contracts t = tq*XT*K + p*XT + sub on
        # partition p; idsrep repeats each id S times along the free dim so
        # one is_equal against iota_t yields all NCHUNK mask chunks
        idsrep = np.repeat(
            ids.reshape(BL, NCHUNK // XT, K, XT).transpose(2, 0, 1, 3)
            .reshape(K, BL, NCHUNK),
            S, axis=2,
        ).reshape(K, BL * NCHUNK * S)
        cst8 = np.empty((K, CW8), dtype=np.int8)
        cst8[:, C8_IDSBC:C8_IDSBC + T] = np.repeat(ids, S, axis=0)
        cst8[:, C8_IDSREP:C8_IDSREP + BL * NCHUNK * S] = idsrep
        cst8[:, C8_IOTAT:C8_IOTAT + NCHUNK * S] = iota_t
        in_maps.append({
            "x": inputs[b0:b0 + BL],
            "cst8": cst8,
            "tmt16": tmt16,
            "cst32": cst32,
        })
    return in_maps


def finish_host(out5_per_core: list[np.ndarray], binary_labels: np.ndarray):
    """Combine per-sample (npairs, n, ninv, S1, S2) with labels."""
    v = np.concatenate([np.asarray(o, np.float64) for o in out5_per_core], axis=0)
    npairs, n, ninv, s1, s2 = v[:, 0], v[:, 1], v[:, 2], v[:, 3], v[:, 4]
    labels = np.asarray(binary_labels)
    loss_pos = s1 / np.maximum(npairs, 1.0)
    loss_neg = s2 / np.maximum(ninv, 1.0)
    pos_count = (labels == 1) & (n >= 2)
    neg_count = (labels == 0) & (ninv > 0)
    total = (loss_pos * pos_count).sum() + (loss_neg * neg_count).sum()
    num = pos_count.sum() + neg_count.sum()
    return np.float32(total / (num + 1e-9))


def kernel(inputs, step_ids, binary_labels, _trace=False):
    nc = get_program()
    in_maps = make_in_maps(inputs, step_ids)
    res = run_bass_kernel_spmd(
        nc, in_maps, core_ids=list(range(N_CORES)), trace=_trace
    )
    out = finish_host([r["out5"] for r in res.results], binary_labels)
    if _trace:
        return out, res
    return out



# revision 5
# speedup vs baseline: 1.1484x; 1.1484x over previous
"""Trainium2 Bass kernel for nn_MaxMarginLoss (segment_reduce).

Data-parallel over the batch: 32 samples -> 8 NeuronCores x 4 samples.

Everything derivable from step_ids (segment counts, first-appearance
order, the adjacent-pair adjacency A, pair validity) is integer work on
a [B,T] int tensor -- precomputed on the host, like the baseline's mask
prep.  That lets the whole per-sample pipeline fold into the streaming
matmul: with G = (I - A) @ diag(recip/sqrt(D)) (per sample) and
mask_c the per-chunk one-hot step mask,

    diff = (I - A) @ diag(r) @ segsum(|x|) = sum_c (G @ mask_c^T) @ |x_c|
         = sum_c W_c @ |x_c|

so the device just streams x (the 32 MiB/core memory-bound part),
takes |x| in bf16, and accumulates one matmul per chunk-half directly
into a per-sample PSUM `diff` tile -- no segment-sum PSUM, no scale
copy, no reorder matmul, no subtract.  Per sample the tail is a single
fused relu(diff)^2 free-dim-accumulate plus three tiny vector ops; the
host applies labels and the final scalar division.

W_c in bf16 only perturbs diff by ~2^-9 relative on the *variance*
part of H (the mean component cancels in H_i - H_next), well inside
the 2e-2 tolerance (measured ~1e-4).
"""

import numpy as np

import concourse.bass as bass
from concourse import mybir
from concourse.bass_utils import run_bass_kernel_spmd
from concourse.tile import TileContext
from concourse.vector_clock import ScopedClock

F32 = mybir.dt.float32
BF16 = mybir.dt.bfloat16
U16 = mybir.dt.uint16
OP = mybir.AluOpType
AF = mybir.ActivationFunctionType

B, T, D = 32, 2048, 1024
S = 32          # step ids 1..32; id 0 is padding
ALPHA = 1.0
N_CORES = 8
BL = B // N_CORES           # samples per core
K = 128                     # matmul contraction tile (partitions)
NCHUNK = T // K             # 16 K-chunks per sample
H2 = D // 2

# Per-sample DMA tiling: 7 transfers of 2 chunks (1 MiB) then 2 single-
# chunk transfers (512 KiB).  The tapered last tiles shrink the exposed
# critical path after the final byte lands (CAST + 2 matmuls instead of
# double that).
TILES = [(2 * i, 2) for i in range(7)] + [(14, 1), (15, 1)]


def chunk_tmap(c: int) -> np.ndarray:
    """t index per (partition, sub) for chunk c, matching the DMA APs."""
    p = np.arange(K)
    if c < 14:
        return (c // 2) * 256 + 2 * p + (c % 2)
    return c * K + p


# The public neuronxcc walrus (setupSyncWait in CoreV2/V3GenImpl) only
# supports a small number of embedded semaphore waits per instruction,
# while Tile's scheduler attaches one wait per required logical proc.
# After scheduling, hoist overflow waits onto same-engine no-ops placed
# immediately before the owning instruction: engine program order makes
# that semantically identical.
_MAX_WAITS_DEFAULT = 1
_MAX_WAITS_BY_OPCODE = {}


class _LeanTailTileContext(TileContext):
    """Tile's default kernel tail is drain -> barrier -> sem-clear ->
    barrier.  After the first all-engine barrier no engine can still be
    waiting on a kernel semaphore, so the clears need no cross-engine
    ordering and the second (~3-4 us) barrier can be dropped; each
    engine's stream still ends after its own clears, so re-execution
    sees zeroed semaphores."""

    def _drain_and_barrier(self, tick_clock, wait_clock):
        drain_inst = self.nc.sync.drain()
        wait_clock.add_sem_waits(
            drain_inst.ins, ScopedClock({None: tick_clock.global_clock})
        )
        self.nc.all_engine_barrier()
        assert self.sems is not None
        popped = self.nc._tile_sem_poison_stack.pop()
        assert popped is self._sem_poison
        self.nc.clear_and_free_semaphores(list(self.sems.allocated().values()))


def _split_sync_waits(nc: bass.Bass):
    for f in nc.m.functions:
        for bb in f.blocks:
            insts = list(bb.instructions)
            need = []  # (ins, overflow_waits)
            for ins in insts:
                si = getattr(ins, "sync_info", None)
                if si is None or not si.on_wait:
                    continue
                cap = _MAX_WAITS_BY_OPCODE.get(ins.opcode, _MAX_WAITS_DEFAULT)
                waits = list(si.on_wait)
                if len(waits) <= cap:
                    continue
                ins.sync_info = mybir.SyncInfo(
                    on_wait=waits[:cap], on_update=list(si.on_update)
                )
                need.append((ins, waits[cap:], cap))
            if not need:
                continue
            nop_for: dict[str, list] = {}
            for ins, overflow, cap in need:
                eng = nc.engines[ins.engine]
                nops = []
                for i in range(0, len(overflow), cap):
                    nop = eng.nop(hint="waitsplit", nofuse=True)
                    nop.ins.sync_info = mybir.SyncInfo(
                        on_wait=overflow[i:i + cap], on_update=[]
                    )
                    nops.append(nop.ins)
                nop_for[ins.name] = nops
            created = {n.name for nops in nop_for.values() for n in nops}
            # nop() appended the new instructions to the current bb; pull
            # them out of every block and splice before their owners.
            for bb2 in f.blocks:
                cur = [i for i in bb2.instructions if i.name not in created]
                out = []
                for ins in cur:
                    out.extend(nop_for.get(ins.name, ()))
                    out.append(ins)
                bb2.instructions = out


def build_program() -> bass.Bass:
    nc = bass.Bass()

    x = nc.declare_dram_parameter("x", [BL, T, D], F32, isOutput=False)
    # W_c lhsT blocks for every (sample, chunk): bf16 bit patterns.
    wt16 = nc.declare_dram_parameter(
        "wt16", [K, BL * NCHUNK * S], U16, isOutput=False
    )
    # col 0: succ (pair-valid per step row), col 1: inv (invalid-topology
    # pair), cols 2..5: block-diagonal ones for the per-sample column sum.
    cf = nc.declare_dram_parameter("cf", [K, 6], F32, isOutput=False)
    out2 = nc.declare_dram_parameter("out2", [BL, 2], F32, isOutput=True)

    with _LeanTailTileContext(nc) as tc:
        with (
            tc.tile_pool(name="const", bufs=1) as cpool,
            tc.tile_pool(name="persist", bufs=1) as pp,
            tc.tile_pool(name="xin2", bufs=8) as xin2,
            tc.tile_pool(name="xin1", bufs=4) as xin1,
            tc.tile_pool(name="xa2", bufs=8) as xa2,
            tc.tile_pool(name="xa1", bufs=4) as xa1,
            tc.tile_pool(name="ps", bufs=1, space="PSUM") as psp,
        ):
            # Constants ride the same sync (HWDGE) ring as x, issued
            # first: W lands before the first matmul needs it and the
            # gpsimd/SWDGE path (slow descriptor emission) stays unused,
            # so its drain sems disappear from the kernel tail.
            sb_wt = cpool.tile([K, BL * NCHUNK * S], U16)
            nc.sync.dma_start(out=sb_wt[:], in_=wt16[:])
            sb_cf = cpool.tile([K, 6], F32)
            nc.sync.dma_start(out=sb_cf[:], in_=cf[:])
            sb_succ = sb_cf[:, 0:1]
            sb_inv = sb_cf[:, 1:2]
            sb_bones = sb_cf[:, 2:6]

            # diff accumulates across all 16 chunks of each sample in
            # rows [32b, 32b+32); samples use disjoint partition groups
            # so one 2-bank tile serves all four.
            diff = psp.tile([K, D], F32)
            vp = psp.tile([BL, 8], F32)
            relu_sb = pp.tile([K, D], F32)
            sq = pp.tile([K, D], F32)
            e2 = pp.tile([K, 2], F32)
            e_raw = pp.tile([K, 1], F32)
            ae = pp.tile([K, 1], F32)
            v_t = pp.tile([K, 2], F32)

            # Emitted one sample late so the DVE ops land in the queue
            # after their dependencies are met (emitted eagerly they
            # head-of-line-block the abs stream behind sample b's last
            # matmul and stall the DMAs).
            def sample_tail(b):
                bs = slice(b * S, (b + 1) * S)
                # E_i = sum_d relu(diff)^2 (the 1/D mean and recip are
                # folded into W).  DVE can't read both multiplicands
                # from PSUM, so ACT takes the relu (PSUM -> SBUF, per
                # d-half so the last sample pipelines against the final
                # matmuls) and DVE squares with the free-dim sum fused
                # in (max-with-0 is an identity on relu'd values).
                for h in range(2):
                    hs = slice(h * H2, (h + 1) * H2)
                    nc.scalar.activation(
                        relu_sb[bs, hs], diff[bs, hs], AF.Relu
                    )
                for h in range(2):
                    hs = slice(h * H2, (h + 1) * H2)
                    nc.vector.scalar_tensor_tensor(
                        sq[bs, hs], relu_sb[bs, hs], 0.0, relu_sb[bs, hs],
                        op0=OP.max, op1=OP.mult,
                        accum_out=e2[bs, h:h + 1],
                    )
                nc.vector.tensor_tensor(
                    e_raw[bs, :], e2[bs, 0:1], e2[bs, 1:2], OP.add
                )
                nc.vector.tensor_tensor(
                    v_t[bs, 0:1], e_raw[bs, :], sb_succ[bs, :], OP.mult
                )
                nc.vector.tensor_scalar(
                    ae[bs, :], e_raw[bs, :], -1.0, ALPHA, OP.mult, OP.add
                )
                nc.vector.tensor_scalar(
                    v_t[bs, 1:2], ae[bs, :], 0.0, sb_inv[bs, :],
                    OP.max, OP.mult,
                )

            ti = 0
            for b in range(BL):
                for tix, (c0, xt) in enumerate(TILES):
                    if b > 0 and tix == 2:
                        sample_tail(b - 1)
                    xpool, apool = (xin2, xa2) if xt == 2 else (xin1, xa1)
                    xtile = xpool.tile([K, xt, D], F32)
                    # All x DMAs on the sync ring: one HWDGE ring's
                    # descriptor feed saturates all 16 SDMA engines with
                    # >=4 KiB/partition contiguous rows.
                    if xt == 2:
                        src = x[b, c0 * K:(c0 + 2) * K, :].rearrange(
                            "(p s) d -> p s d", p=K
                        )
                    else:
                        src = x[b, c0 * K:(c0 + 1) * K, :].rearrange(
                            "(p s) d -> p s d", p=K
                        )
                    nc.sync.dma_start(out=xtile[:], in_=src)

                    # |x| rounded to bf16: PE runs bf16 at 1 cycle/row vs
                    # fp32's 4; the 2^-9 rounding washes out in the loss.
                    # Alternate engines: ACT computes Abs->bf16 directly;
                    # DVE casts (RNE, so |bf16(x)| == bf16(|x|)) then
                    # clears the sign bit in the 16-bit 4x mode.
                    xa = apool.tile([K, xt, D], BF16)
                    last_tile = b == BL - 1 and tix == len(TILES) - 1
                    if last_tile:
                        # Split the final CAST across both engines so
                        # each matmul half starts as soon as possible.
                        nc.scalar.activation(
                            xa[:, 0, 0:H2], xtile[:, 0, 0:H2], AF.Abs
                        )
                        nc.vector.tensor_copy(
                            xa[:, 0, H2:D], xtile[:, 0, H2:D]
                        )
                        nc.vector.tensor_scalar(
                            xa[:, 0, H2:D].bitcast(U16),
                            xa[:, 0, H2:D].bitcast(U16),
                            0x7FFF, None, OP.bitwise_and,
                        )
                    elif ti % 2 == 0:
                        nc.scalar.activation(xa[:], xtile[:], AF.Abs)
                    else:
                        nc.vector.tensor_copy(xa[:], xtile[:])
                        nc.vector.tensor_scalar(
                            xa[:].bitcast(U16), xa[:].bitcast(U16),
                            0x7FFF, None, OP.bitwise_and,
                        )
                    ti += 1

                    for sub in range(xt):
                        c = c0 + sub
                        wcol = (b * NCHUNK + c) * S
                        for h in range(2):
                            nc.tensor.matmul(
                                diff[b * S:(b + 1) * S, h * H2:(h + 1) * H2],
                                lhsT=sb_wt[:, wcol:wcol + S].bitcast(BF16),
                                rhs=xa[:, sub, h * H2:(h + 1) * H2],
                                start=(c == 0), stop=(c == NCHUNK - 1),
                                tile_position=(0, b * S),
                            )

            sample_tail(BL - 1)

            # per-sample column sums: blockones[128,4].T @ V[128,2] -> [4,2]
            nc.tensor.matmul(
                vp[:, 0:2], lhsT=sb_bones[:], rhs=v_t[:, 0:2],
                start=True, stop=True,
            )
            out_sb = pp.tile([BL, 2], F32)
            nc.vector.tensor_copy(out_sb[:], vp[:, 0:2])
            nc.sync.dma_start(out=out2[:], in_=out_sb[:])

    _split_sync_waits(nc)
    return nc


_PROGRAM: bass.Bass | None = None


def get_program() -> bass.Bass:
    global _PROGRAM
    if _PROGRAM is None:
        _PROGRAM = build_program()
    return _PROGRAM


def _f32_to_bf16_bits(a: np.ndarray) -> np.ndarray:
    """Round-to-nearest-even f32 -> bf16 bit patterns (uint16)."""
    u = np.ascontiguousarray(a, dtype=np.float32).view(np.uint32)
    rnd = ((u >> 16) & 1) + np.uint32(0x7FFF)
    return ((u + rnd) >> 16).astype(np.uint16)


def host_prep(step_ids: np.ndarray):
    """Per-sample index math (all integer work on step_ids) plus the
    per-chunk W lhsT blocks.  Returns (per-core in_map extras, per-sample
    scalars for the final host combine)."""
    step_ids = np.asarray(step_ids)
    rsqrt_d = 1.0 / np.sqrt(np.float64(D))

    wt_all = np.empty((B, NCHUNK, K, S), dtype=np.float32)
    succ_all = np.empty((B, S), dtype=np.float32)
    inv_all = np.empty((B, S), dtype=np.float32)
    npairs_all = np.empty(B, dtype=np.int64)
    n_all = np.empty(B, dtype=np.int64)
    ninv_all = np.empty(B, dtype=np.int64)

    steps = np.arange(1, S + 1)
    tmaps = np.stack([chunk_tmap(c) for c in range(NCHUNK)])  # [NCHUNK, K]

    for gb in range(B):
        ids = step_ids[gb]                                   # [T]
        mask = ids[:, None] == steps[None, :]                # [T, S]
        counts = mask.sum(axis=0)
        recip = 1.0 / np.maximum(counts, 1.0)
        pos = np.where(mask, np.arange(T)[:, None], T).min(axis=0)
        perm = np.argsort(pos, kind="stable")
        ordered_steps = steps[perm]
        present_slot = pos[perm] < T
        n = int(present_slot.sum())

        # row i = step id i+1; rank = slot index in appearance order
        rank = np.empty(S, dtype=np.int64)
        rank[perm] = np.arange(S)

        succ = np.zeros(S, dtype=np.float32)
        inv = np.zeros(S, dtype=np.float32)
        G = np.zeros((S, S), dtype=np.float64)
        for i in range(S):
            k = rank[i]
            if k + 1 < S and present_slot[k] and present_slot[k + 1]:
                nxt = perm[k + 1]
                succ[i] = 1.0
                if ordered_steps[k] > ordered_steps[k + 1]:
                    inv[i] = 1.0
                G[i, i] = recip[i] * rsqrt_d
                G[i, nxt] -= recip[nxt] * rsqrt_d

        npairs_all[gb] = int(succ.sum())
        n_all[gb] = n
        ninv_all[gb] = int(inv.sum())
        succ_all[gb] = succ
        inv_all[gb] = inv

        # W_c^T[p, i] = G[i, s_p] for the step s_p at t = tmap(c, p)
        ids_c = ids[tmaps]                                   # [NCHUNK, K]
        valid = ids_c >= 1
        gidx = np.clip(ids_c - 1, 0, S - 1)
        wt_all[gb] = np.where(
            valid[:, :, None], G.T[gidx, :], 0.0
        ).astype(np.float32)

    return wt_all, succ_all, inv_all, npairs_all, n_all, ninv_all


def make_in_maps(inputs: np.ndarray, step_ids: np.ndarray):
    inputs = np.ascontiguousarray(np.asarray(inputs, dtype=np.float32))
    wt_all, succ_all, inv_all, npairs, n, ninv = host_prep(step_ids)

    bones = ((np.arange(K)[:, None] // S) == np.arange(BL)[None, :]).astype(
        np.float32
    )

    in_maps = []
    for core in range(N_CORES):
        b0 = core * BL
        # [K, BL*NCHUNK*S] with the (sample, chunk) blocks contiguous
        wt = wt_all[b0:b0 + BL].transpose(2, 0, 1, 3).reshape(K, -1)
        cf = np.empty((K, 6), dtype=np.float32)
        cf[:, 0] = succ_all[b0:b0 + BL].reshape(K)
        cf[:, 1] = inv_all[b0:b0 + BL].reshape(K)
        cf[:, 2:6] = bones
        in_maps.append({
            "x": inputs[b0:b0 + BL],
            "wt16": _f32_to_bf16_bits(wt),
            "cf": cf,
        })
    return in_maps, (npairs, n, ninv)


def finish_host(out2_per_core, stats, binary_labels) -> np.float32:
    npairs, n, ninv = stats
    v = np.concatenate(
        [np.asarray(o, np.float64) for o in out2_per_core], axis=0
    )
    s1, s2 = v[:, 0], v[:, 1]
    labels = np.asarray(binary_labels)
    loss_pos = s1 / np.maximum(npairs, 1.0)
    loss_neg = s2 / np.maximum(ninv, 1.0)
    pos_count = (labels == 1) & (n >= 2)
    neg_count = (labels == 0) & (ninv > 0)
    total = (loss_pos * pos_count).sum() + (loss_neg * neg_count).sum()
    num = pos_count.sum() + neg_count.sum()
    return np.float32(total / (num + 1e-9))


def kernel(inputs, step_ids, binary_labels, _trace=False):
    nc = get_program()
    in_maps, stats = make_in_maps(inputs, step_ids)
    res = run_bass_kernel_spmd(
        nc, in_maps, core_ids=list(range(N_CORES)), trace=_trace
    )
    out = finish_host([r["out2"] for r in res.results], stats, binary_labels)
    if _trace:
        return out, res
    return out


# revision 7
# speedup vs baseline: 1.2967x; 1.1291x over previous
"""Trainium2 Bass kernel for nn_MaxMarginLoss (segment_reduce).

Data-parallel over the batch: 32 samples -> 8 NeuronCores x 4 samples.

Everything derivable from step_ids (segment counts, first-appearance
order, the adjacent-pair adjacency A, pair validity) is integer work on
a [B,T] int tensor -- precomputed on the host, like the baseline's mask
prep.  That lets the whole per-sample pipeline fold into the streaming
matmul: with G = (I - A) @ diag(recip/sqrt(D)) (per sample) and
mask_c the per-chunk one-hot step mask,

    diff = (I - A) @ diag(r) @ segsum(|x|) = sum_c (G @ mask_c^T) @ |x_c|
         = sum_c W_c @ |x_c|

so the device just streams x (the 32 MiB/core memory-bound part),
takes |x| in bf16, and accumulates one matmul per chunk-half directly
into a per-sample PSUM `diff` tile -- no segment-sum PSUM, no scale
copy, no reorder matmul, no subtract.  Per sample the tail is an ACT
relu (DVE cannot read two PSUM operands) plus a fused square-accumulate
and two tiny vector ops; the succ/inv pair weights ride in the final
column-sum matmul's lhsT; the host applies labels and the final scalar
division.

W_c in bf16 only perturbs diff by ~2^-9 relative on the *variance*
part of H (the mean component cancels in H_i - H_next), well inside
the 2e-2 tolerance (measured ~1e-5).
"""

import numpy as np

import concourse.bass as bass
from concourse import mybir
from concourse.bass_utils import run_bass_kernel_spmd
from concourse.tile import TileContext
from concourse.vector_clock import ScopedClock

F32 = mybir.dt.float32
BF16 = mybir.dt.bfloat16
U16 = mybir.dt.uint16
OP = mybir.AluOpType
AF = mybir.ActivationFunctionType

B, T, D = 32, 2048, 1024
S = 32          # step ids 1..32; id 0 is padding
ALPHA = 1.0
N_CORES = 8
BL = B // N_CORES           # samples per core
K = 128                     # matmul contraction tile (partitions)
NCHUNK = T // K             # 16 K-chunks per sample
H2 = D // 2

# Per-sample DMA tiling: 16 KiB contiguous DRAM per partition (XT=4)
# runs the SDMA engines closer to their ~27 GB/s streaming rate than
# 8 KiB rows; the tapered tail tiles shrink the exposed critical path
# after the final byte lands.
TILES = [(0, 4), (4, 4), (8, 4), (12, 2), (14, 1), (15, 1)]


def chunk_tmap(c: int) -> np.ndarray:
    """t index per (partition, sub) for chunk c, matching the DMA APs."""
    p = np.arange(K)
    if c < 12:
        return (c // 4) * 512 + 4 * p + (c % 4)
    if c < 14:
        return 1536 + 2 * p + (c - 12)
    return c * K + p


# The public neuronxcc walrus (setupSyncWait in CoreV2/V3GenImpl) only
# supports a small number of embedded semaphore waits per instruction,
# while Tile's scheduler attaches one wait per required logical proc.
# After scheduling, hoist overflow waits onto same-engine no-ops placed
# immediately before the owning instruction: engine program order makes
# that semantically identical.
_MAX_WAITS_DEFAULT = 1
_MAX_WAITS_BY_OPCODE = {}


class _LeanTailTileContext(TileContext):
    """Tile's default kernel tail is drain -> barrier -> sem-clear ->
    barrier.  After the first all-engine barrier no engine can still be
    waiting on a kernel semaphore, so the clears need no cross-engine
    ordering and the second (~3-4 us) barrier can be dropped; each
    engine's stream still ends after its own clears, so re-execution
    sees zeroed semaphores."""

    def _drain_and_barrier(self, tick_clock, wait_clock):
        drain_inst = self.nc.sync.drain()
        wait_clock.add_sem_waits(
            drain_inst.ins, ScopedClock({None: tick_clock.global_clock})
        )
        self.nc.all_engine_barrier()
        assert self.sems is not None
        popped = self.nc._tile_sem_poison_stack.pop()
        assert popped is self._sem_poison
        self.nc.clear_and_free_semaphores(list(self.sems.allocated().values()))


def _split_sync_waits(nc: bass.Bass):
    for f in nc.m.functions:
        for bb in f.blocks:
            insts = list(bb.instructions)
            need = []  # (ins, overflow_waits)
            for ins in insts:
                si = getattr(ins, "sync_info", None)
                if si is None or not si.on_wait:
                    continue
                cap = _MAX_WAITS_BY_OPCODE.get(ins.opcode, _MAX_WAITS_DEFAULT)
                waits = list(si.on_wait)
                if len(waits) <= cap:
                    continue
                ins.sync_info = mybir.SyncInfo(
                    on_wait=waits[:cap], on_update=list(si.on_update)
                )
                need.append((ins, waits[cap:], cap))
            if not need:
                continue
            nop_for: dict[str, list] = {}
            for ins, overflow, cap in need:
                eng = nc.engines[ins.engine]
                nops = []
                for i in range(0, len(overflow), cap):
                    nop = eng.nop(hint="waitsplit", nofuse=True)
                    nop.ins.sync_info = mybir.SyncInfo(
                        on_wait=overflow[i:i + cap], on_update=[]
                    )
                    nops.append(nop.ins)
                nop_for[ins.name] = nops
            created = {n.name for nops in nop_for.values() for n in nops}
            # nop() appended the new instructions to the current bb; pull
            # them out of every block and splice before their owners.
            for bb2 in f.blocks:
                cur = [i for i in bb2.instructions if i.name not in created]
                out = []
                for ins in cur:
                    out.extend(nop_for.get(ins.name, ()))
                    out.append(ins)
                bb2.instructions = out


def _strip_constructor_tail(nc: bass.Bass, names: set[str]):
    """Drop the Bass-constructor const-AP memsets (this kernel never
    reads the const APs) and the constructor's all-engine barrier (the
    body's cross-engine ordering is fully semaphore-driven; engine
    streams are self-ordered against their own preamble).  Saves ~1 us
    of serial startup before the first DMA issue."""
    drop_ops = {"Memset", "Drain", "EventSemaphore"}
    for f in nc.m.functions:
        for bb in f.blocks:
            bb.instructions = [
                i for i in bb.instructions
                if not (i.name in names and i.opcode in drop_ops)
            ]


def build_program() -> bass.Bass:
    nc = bass.Bass()
    ctor_names = {
        i.name for f in nc.m.functions for bb in f.blocks
        for i in bb.instructions
    }

    x = nc.declare_dram_parameter("x", [BL, T, D], F32, isOutput=False)
    # W_c lhsT blocks for every (sample, chunk): bf16 bit patterns.
    wt16 = nc.declare_dram_parameter(
        "wt16", [K, BL * NCHUNK * S], U16, isOutput=False
    )
    # cols 0-3: succ * block-diag ones, cols 4-7: inv * block-diag ones
    # (lhsT of the final column-sum matmul).
    cf = nc.declare_dram_parameter("cf", [K, 8], F32, isOutput=False)
    out3 = nc.declare_dram_parameter("out3", [8, 3], F32, isOutput=True)

    with _LeanTailTileContext(nc) as tc:
        with (
            tc.tile_pool(name="const", bufs=1) as cpool,
            tc.tile_pool(name="persist", bufs=1) as pp,
            tc.tile_pool(name="xin4", bufs=3) as xin4,
            tc.tile_pool(name="xin2", bufs=2) as xin2,
            tc.tile_pool(name="xin1", bufs=3) as xin1,
            tc.tile_pool(name="xa4", bufs=3) as xa4,
            tc.tile_pool(name="xa2", bufs=2) as xa2,
            tc.tile_pool(name="xa1", bufs=3) as xa1,
            tc.tile_pool(name="ps", bufs=1, space="PSUM") as psp,
        ):
            sb_wt = cpool.tile([K, BL * NCHUNK * S], U16)
            sb_cf = cpool.tile([K, 8], F32)

            # diff accumulates across all 16 chunks of each sample in
            # rows [32b, 32b+32); samples use disjoint partition groups
            # so one 2-bank tile serves all four.
            diff = psp.tile([K, D], F32)
            vp = psp.tile([8, 8], F32)
            relu_sb = pp.tile([K, D], BF16)
            sq = pp.tile([K, D], BF16)
            er3 = pp.tile([K, 3], F32)   # e_half0, e_half1, relu(1-E)
            e_raw = pp.tile([K, 1], F32)
            ae = pp.tile([K, 1], F32)

            # Emitted one sample late so the ACT/DVE ops land in their
            # queues after their dependencies are met (emitted eagerly
            # they head-of-line-block the abs stream behind sample b's
            # last matmul and stall the DMAs).
            def sample_tail(b):
                bs = slice(b * S, (b + 1) * S)
                # E_i = sum_d relu(diff)^2 (the 1/D mean and recip are
                # folded into W).  DVE can't read both multiplicands
                # from PSUM, so ACT takes the relu (PSUM -> SBUF bf16,
                # per d-half so the last sample pipelines against the
                # final matmuls) and DVE squares at the 16-bit rate with
                # the free-dim sum fused in (max-with-0 is an identity
                # on relu'd values).
                for h in range(2):
                    hs = slice(h * H2, (h + 1) * H2)
                    nc.scalar.activation(
                        relu_sb[bs, hs], diff[bs, hs], AF.Relu
                    )
                for h in range(2):
                    hs = slice(h * H2, (h + 1) * H2)
                    nc.vector.scalar_tensor_tensor(
                        sq[bs, hs], relu_sb[bs, hs], 0.0, relu_sb[bs, hs],
                        op0=OP.max, op1=OP.mult,
                        accum_out=er3[bs, h:h + 1],
                    )
                nc.vector.tensor_tensor(
                    e_raw[bs, :], er3[bs, 0:1], er3[bs, 1:2], OP.add
                )
                nc.vector.tensor_scalar(
                    ae[bs, :], e_raw[bs, :], -1.0, ALPHA, OP.mult, OP.add
                )
                nc.vector.tensor_scalar(
                    er3[bs, 2:3], ae[bs, :], 0.0, None, OP.max
                )

            ti = 0
            for b in range(BL):
                for tix, (c0, xt) in enumerate(TILES):
                    if b > 0 and tix == 1:
                        sample_tail(b - 1)
                    xpool, apool = {
                        4: (xin4, xa4), 2: (xin2, xa2), 1: (xin1, xa1)
                    }[xt]
                    xtile = xpool.tile([K, xt, D], F32)
                    # All x DMAs on the sync ring: one HWDGE ring's
                    # descriptor feed saturates all 16 SDMA engines with
                    # >=4 KiB/partition contiguous rows.
                    src = x[b, c0 * K:(c0 + xt) * K, :].rearrange(
                        "(p s) d -> p s d", p=K
                    )
                    nc.sync.dma_start(out=xtile[:], in_=src)
                    if b == 0 and tix == 0:
                        # Constants follow the first x tile on the same
                        # ring: the stream's first byte lands ~1.3 us
                        # earlier than if W went first, and W still
                        # arrives before the first matmul wants it.
                        nc.sync.dma_start(out=sb_wt[:], in_=wt16[:])
                        nc.sync.dma_start(out=sb_cf[:], in_=cf[:])

                    # |x| rounded to bf16: PE runs bf16 at 1 cycle/row
                    # vs fp32's 4; the 2^-9 rounding washes out in the
                    # loss.  Both engines split every tile (ACT: Abs
                    # activation; DVE: abs_max with 0) so neither backs
                    # up near the stream's end.
                    xa = apool.tile([K, xt, D], BF16)
                    last_tile = b == BL - 1 and tix == len(TILES) - 1
                    def dve_abs(dst, srcap):
                        # DVE abs: cast f32->bf16 (RNE, so |bf16(x)| ==
                        # bf16(|x|)) then clear the sign bit in the
                        # 16-bit 4x mode.
                        nc.vector.tensor_copy(dst, srcap)
                        nc.vector.tensor_scalar(
                            dst.bitcast(U16), dst.bitcast(U16),
                            0x7FFF, None, OP.bitwise_and,
                        )

                    if xt == 4:
                        nc.scalar.activation(
                            xa[:, 0:2, :], xtile[:, 0:2, :], AF.Abs
                        )
                        dve_abs(xa[:, 2:4, :], xtile[:, 2:4, :])
                    elif xt == 2:
                        nc.scalar.activation(
                            xa[:, 0, :], xtile[:, 0, :], AF.Abs
                        )
                        dve_abs(xa[:, 1, :], xtile[:, 1, :])
                    elif last_tile:
                        # Split the final tile's abs across both engines
                        # so each matmul half starts as soon as possible.
                        nc.scalar.activation(
                            xa[:, 0, 0:H2], xtile[:, 0, 0:H2], AF.Abs
                        )
                        dve_abs(xa[:, 0, H2:D], xtile[:, 0, H2:D])
                    elif ti % 2 == 0:
                        nc.scalar.activation(xa[:], xtile[:], AF.Abs)
                    else:
                        dve_abs(xa[:], xtile[:])
                    ti += 1

                    for sub in range(xt):
                        c = c0 + sub
                        wcol = (b * NCHUNK + c) * S
                        for h in range(2):
                            nc.tensor.matmul(
                                diff[b * S:(b + 1) * S, h * H2:(h + 1) * H2],
                                lhsT=sb_wt[:, wcol:wcol + S].bitcast(BF16),
                                rhs=xa[:, sub, h * H2:(h + 1) * H2],
                                start=(c == 0), stop=(c == NCHUNK - 1),
                                tile_position=(0, b * S),
                            )

            sample_tail(BL - 1)

            # s1/s2 column sums with succ/inv folded into the lhsT:
            # out[b, 0] + out[b, 1] = sum_i succ_i E_i (halves), and
            # out[4+b, 2] = sum_i inv_i relu(1-E_i).
            nc.tensor.matmul(
                vp[:, 0:3], lhsT=sb_cf[:], rhs=er3[:],
                start=True, stop=True,
            )
            out_sb = pp.tile([8, 3], F32)
            nc.vector.tensor_copy(out_sb[:], vp[:, 0:3])
            nc.sync.dma_start(out=out3[:], in_=out_sb[:])

    _split_sync_waits(nc)
    _strip_constructor_tail(nc, ctor_names)
    return nc


_PROGRAM: bass.Bass | None = None


def get_program() -> bass.Bass:
    global _PROGRAM
    if _PROGRAM is None:
        _PROGRAM = build_program()
    return _PROGRAM


def _f32_to_bf16_bits(a: np.ndarray) -> np.ndarray:
    """Round-to-nearest-even f32 -> bf16 bit patterns (uint16)."""
    u = np.ascontiguousarray(a, dtype=np.float32).view(np.uint32)
    rnd = ((u >> 16) & 1) + np.uint32(0x7FFF)
    return ((u + rnd) >> 16).astype(np.uint16)


def host_prep(step_ids: np.ndarray):
    """Per-sample index math (all integer work on step_ids) plus the
    per-chunk W lhsT blocks.  Returns (per-core in_map extras, per-sample
    scalars for the final host combine)."""
    step_ids = np.asarray(step_ids)
    rsqrt_d = 1.0 / np.sqrt(np.float64(D))

    wt_all = np.empty((B, NCHUNK, K, S), dtype=np.float32)
    succ_all = np.empty((B, S), dtype=np.float32)
    inv_all = np.empty((B, S), dtype=np.float32)
    npairs_all = np.empty(B, dtype=np.int64)
    n_all = np.empty(B, dtype=np.int64)
    ninv_all = np.empty(B, dtype=np.int64)

    steps = np.arange(1, S + 1)
    tmaps = np.stack([chunk_tmap(c) for c in range(NCHUNK)])  # [NCHUNK, K]

    for gb in range(B):
        ids = step_ids[gb]                                   # [T]
        mask = ids[:, None] == steps[None, :]                # [T, S]
        counts = mask.sum(axis=0)
        recip = 1.0 / np.maximum(counts, 1.0)
        pos = np.where(mask, np.arange(T)[:, None], T).min(axis=0)
        perm = np.argsort(pos, kind="stable")
        ordered_steps = steps[perm]
        present_slot = pos[perm] < T
        n = int(present_slot.sum())

        # row i = step id i+1; rank = slot index in appearance order
        rank = np.empty(S, dtype=np.int64)
        rank[perm] = np.arange(S)

        succ = np.zeros(S, dtype=np.float32)
        inv = np.zeros(S, dtype=np.float32)
        G = np.zeros((S, S), dtype=np.float64)
        for i in range(S):
            k = rank[i]
            if k + 1 < S and present_slot[k] and present_slot[k + 1]:
                nxt = perm[k + 1]
                succ[i] = 1.0
                if ordered_steps[k] > ordered_steps[k + 1]:
                    inv[i] = 1.0
                G[i, i] = recip[i] * rsqrt_d
                G[i, nxt] -= recip[nxt] * rsqrt_d

        npairs_all[gb] = int(succ.sum())
        n_all[gb] = n
        ninv_all[gb] = int(inv.sum())
        succ_all[gb] = succ
        inv_all[gb] = inv

        # W_c^T[p, i] = G[i, s_p] for the step s_p at t = tmap(c, p)
        ids_c = ids[tmaps]                                   # [NCHUNK, K]
        valid = ids_c >= 1
        gidx = np.clip(ids_c - 1, 0, S - 1)
        wt_all[gb] = np.where(
            valid[:, :, None], G.T[gidx, :], 0.0
        ).astype(np.float32)

    return wt_all, succ_all, inv_all, npairs_all, n_all, ninv_all


def make_in_maps(inputs: np.ndarray, step_ids: np.ndarray):
    inputs = np.ascontiguousarray(np.asarray(inputs, dtype=np.float32))
    wt_all, succ_all, inv_all, npairs, n, ninv = host_prep(step_ids)

    bones = ((np.arange(K)[:, None] // S) == np.arange(BL)[None, :]).astype(
        np.float32
    )

    in_maps = []
    for core in range(N_CORES):
        b0 = core * BL
        # [K, BL*NCHUNK*S] with the (sample, chunk) blocks contiguous
        wt = wt_all[b0:b0 + BL].transpose(2, 0, 1, 3).reshape(K, -1)
        cf = np.empty((K, 8), dtype=np.float32)
        cf[:, 0:4] = succ_all[b0:b0 + BL].reshape(K)[:, None] * bones
        cf[:, 4:8] = inv_all[b0:b0 + BL].reshape(K)[:, None] * bones
        in_maps.append({
            "x": inputs[b0:b0 + BL],
            "wt16": _f32_to_bf16_bits(wt),
            "cf": cf,
        })
    return in_maps, (npairs, n, ninv)


def finish_host(out3_per_core, stats, binary_labels) -> np.float32:
    npairs, n, ninv = stats
    s1 = np.concatenate(
        [np.asarray(o, np.float64)[0:4, 0] + np.asarray(o, np.float64)[0:4, 1]
         for o in out3_per_core]
    )
    s2 = np.concatenate(
        [np.asarray(o, np.float64)[4:8, 2] for o in out3_per_core]
    )
    labels = np.asarray(binary_labels)
    loss_pos = s1 / np.maximum(npairs, 1.0)
    loss_neg = s2 / np.maximum(ninv, 1.0)
    pos_count = (labels == 1) & (n >= 2)
    neg_count = (labels == 0) & (ninv > 0)
    total = (loss_pos * pos_count).sum() + (loss_neg * neg_count).sum()
    num = pos_count.sum() + neg_count.sum()
    return np.float32(total / (num + 1e-9))


def kernel(inputs, step_ids, binary_labels, _trace=False):
    nc = get_program()
    in_maps, stats = make_in_maps(inputs, step_ids)
    res = run_bass_kernel_spmd(
        nc, in_maps, core_ids=list(range(N_CORES)), trace=_trace
    )
    out = finish_host([r["out3"] for r in res.results], stats, binary_labels)
    if _trace:
        return out, res
    return out
